# revision 56
# baseline (speedup 1.0000x reference)
"""AUCLoss kernel for 8 TRN2 NeuronCores.

Math: loss = sum_{i,j} pw_i * nw_j * softplus(p_j - p_i) / (n_pos * n_neg)
where pw/nw are per-element weights masked to label==1 / label==0.

Instead of materializing the N x N (13184^2) pairwise matrix, we use a
truncated Fourier expansion of softplus on the diff domain [-12, 12]:

    softplus(x) ~= sum_k a_k cos(w_k x) + b_k sin(w_k x)

which separates over pairs (x = n - p):
    cos(w(n-p)) = cos(wn)cos(wp) + sin(wn)sin(wp)
    sin(w(n-p)) = sin(wn)cos(wp) - cos(wn)sin(wp)

So the whole pairwise sum reduces to weighted Fourier feature sums
    C[w,k] = sum_i w_i cos(w_k p_i),  S[w,k] = sum_i w_i sin(w_k p_i)
for weight vectors in {pos-weight, neg-weight, mask1, mask0}, then a tiny
bilinear combine. The frequencies are snapped to exact-bf16 values and the
coefficients are least-squares fit for those exact frequencies.

Each core processes 1648 points (13 blocks of 128):
  - two bf16 matmuls against a constant block-diagonal frequency matrix
    build all phase arguments w_k * x + phi (phi = pi/2 turns sin into
    cos), one matmul per PSUM bank so DVE and ScalarE never read the
    same bank concurrently (that hangs the core). x is passed as an
    exact 2-way bf16 split (hi/lo) so the bf16 matmul reproduces
    near-fp32 phases in a single pass instead of fp32's LOW/HIGH
    double pass.
  - the hardware Sin spline is only valid on [-pi, pi] (measured:
    garbage beyond |x|~3.2), so the phases are range-reduced first:
    k = round(arg/2pi) via the fp32 magic-number trick, r = arg - 2pi*k.
    Chunk A runs on DVE; chunk B's k comes from ScalarE Identity
    activations so the two engines split the work.
  - two Sin activations produce all 2F=32 bf16 features per point
  - 5 DVE ops build [pw_hi, pw_lo, nw_hi, nw_lo, m1] weight columns
    from a host-side exact bf16 hi/lo split of the sample weights (mask
    products are exactly representable, so the weighted sums are
    fp32-accurate after the hi+lo recombine on the host)
  - 13 accumulating bf16 matmuls contract points -> [5, 32] partial sums
Host sums the 8 per-core [5, 32] partials and applies the combine
(n_neg = N - n_pos since labels are {0,1}).
"""

import os

import numpy as np
import ml_dtypes

import concourse.bass as bass
import concourse.mybir as mybir
from concourse.bass_utils import run_bass_kernel_spmd

# ---------------------------------------------------------------- constants
B, C = 64, 206
N = B * C                      # 13184 flattened preds
NCORES = 8
CHUNK = N // NCORES            # 1648 points per core
BLOCKS = 13                    # ceil(1648 / 128)
PAD = BLOCKS * 128             # 1664 (16 zero-pad points per core)
F = 16                         # frequencies
NFEAT = 2 * F                  # 32 features per point (cos block | sin block)
PERIOD = 12.5                  # nominal half-period of the Fourier basis
FIT_X = 9.0                    # fit domain for softplus diffs
KROWS = 2 * BLOCKS + 2         # 28 lhsT rows: x_hi, x_lo blocks + 2 ones rows
NW = 5                         # weight columns per block (no mask0: n_neg = N - n_pos)
LWCOLS = 3 * BLOCKS + 3        # labels | w_hi | w_lo | zeros | MAGIC | -MAGIC

MAGIC = 12582912.0             # 1.5 * 2^23: fp32 round-to-nearest-int trick
INV2PI = float(np.float32(1.0 / (2 * np.pi)))
NEG2PI = -float(np.float32(2 * np.pi))

_f32 = mybir.dt.float32
_bf16 = mybir.dt.bfloat16
_bf = ml_dtypes.bfloat16


def _bf16_split2(x):
    """Exact 2-way bf16 split: x ~= hi + lo to ~2^-18 relative."""
    x = np.asarray(x, dtype=np.float64)
    hi = x.astype(_bf)
    lo = (x - hi.astype(np.float64)).astype(_bf)
    return hi, lo


def _fit_fourier():
    """Least-squares fit softplus(x) on [-FIT_X, FIT_X] in the basis
    {cos(w_k x), sin(w_k x)} with w_k snapped to exact bf16 values."""
    w = (np.arange(F) * np.pi / PERIOD).astype(_bf).astype(np.float64)
    xs = np.linspace(-FIT_X, FIT_X, 8001)
    A = np.concatenate(
        [np.cos(np.outer(xs, w)), np.sin(np.outer(xs, w))], axis=1
    )
    y = np.log1p(np.exp(-np.abs(xs))) + np.maximum(xs, 0.0)
    coef = np.linalg.solve(A.T @ A + 1e-9 * np.eye(2 * F), A.T @ y)
    return w, coef[:F], coef[F:]


_OMEGA, _COEF_A, _COEF_B = _fit_fourier()


def _mfeat_const():
    """[KROWS, BLOCKS*NFEAT] bf16 frequency matrix.

    Column b*NFEAT + j: frequency _OMEGA[j % F] in rows b and 13+b
    (multiplying x_hi, x_lo of block b), phase pi/2 (2-way bf16 split
    in the two ones-rows) for j < F (cos features), 0 for sin.
    """
    m = np.zeros((KROWS, BLOCKS * NFEAT), dtype=np.float64)
    for b in range(BLOCKS):
        for r in (b, BLOCKS + b):
            m[r, b * NFEAT : b * NFEAT + F] = _OMEGA
            m[r, b * NFEAT + F : (b + 1) * NFEAT] = _OMEGA
    ph_hi = _bf(np.pi / 2)
    ph_lo = _bf(np.pi / 2 - float(ph_hi))
    for b in range(BLOCKS):
        m[2 * BLOCKS, b * NFEAT : b * NFEAT + F] = float(ph_hi)
        m[2 * BLOCKS + 1, b * NFEAT : b * NFEAT + F] = float(ph_lo)
    return m.astype(_bf)


_NC_CACHE = None


def _build_nc():
    # Raw Bass (no TileContext): the pipeline is a short linear chain and
    # explicit semaphores avoid both the per-instruction sync-wait slot
    # limits and Tile's multi-microsecond kernel-tail drain/barrier.
    #
    # Scheduling notes:
    # - the two input DMAs and a dummy Sin (which forces the ~2.7us ACT
    #   table load early) are hoisted into the entry block, ahead of the
    #   framework's start drain+barrier, so DMA latency and the table
    #   load overlap the fixed engine-startup preamble
    # - the start drain+barrier pair is removed: all cross-engine
    #   ordering is carried by this kernel's explicit semaphores
    # - everything is processed in two halves (PSUM banks of 512/320
    #   cols) so PE, DVE and ACT pipeline against each other
    nc = bass.Bass(enable_partition_id=False, monotonic_sem_count=0)
    ncols = BLOCKS * NFEAT  # 416: fits one PSUM bank, one feature matmul
    # xmb: cols 0:128 = x hi/mid/lo blocks + ones rows, cols 128: = freqs
    xmb = nc.declare_dram_parameter("xmb", [KROWS, 128 + ncols], _bf16, isOutput=False)
    lw = nc.declare_dram_parameter("lw", [128, LWCOLS], _bf16, isOutput=False)
    out = nc.declare_dram_parameter("out", [NW, NFEAT], _f32, isOutput=True)

    alu = mybir.AluOpType
    eq = alu.is_equal
    mult = alu.mult
    sin_f = mybir.ActivationFunctionType.Sin
    hoist = []
    # chunk A = blocks 0-6, chunk B = blocks 7-12. Chunk B's phase args go
    # to a second PSUM bank (col offset 512): ACT reads bank 1 while DVE
    # reads bank 0 -- concurrent cross-engine reads of one PSUM bank hang.
    SPLIT = 7 * NFEAT  # 224
    BOFF = 512 - SPLIT  # bank-1 offset of chunk B inside the psum tensor
    halves = [(0, SPLIT), (SPLIT, ncols)]

    with (
        nc.sbuf_tensor([KROWS, 128 + ncols], _bf16) as xm_t,
        nc.sbuf_tensor([128, LWCOLS], _bf16) as lw_t,
        nc.sbuf_tensor([128, ncols], _f32) as kred,
        nc.sbuf_tensor([128, ncols], _f32) as rred,
        nc.sbuf_tensor([128, ncols], _bf16) as feat,
        nc.sbuf_tensor([128, NW * BLOCKS], _bf16) as w6,
        nc.sbuf_tensor([128, 1], _f32) as scratch,
        nc.sbuf_tensor([1, 1], _bf16) as scratch_bf,
        nc.sbuf_tensor([NW, NFEAT], _f32) as out_t,
        nc.psum_tensor([128, 512 + ncols - SPLIT], _f32) as arg,
        nc.psum_tensor([NW, NFEAT], _f32) as red,
        nc.semaphore() as dma_x,
        nc.semaphore() as dma_l,
        nc.semaphore() as s_pe,
        nc.semaphore() as s_act,
        nc.semaphore() as s_dve,
        nc.Block(no_gpsimd_drain=True) as block,
    ):
        zeros_col = lw_t[:, 3 * BLOCKS : 3 * BLOCKS + 1]
        magic_col = lw_t[:, 3 * BLOCKS + 1 : 3 * BLOCKS + 2]
        nmagic_col = lw_t[:, 3 * BLOCKS + 2 : 3 * BLOCKS + 3]
        ident = mybir.ActivationFunctionType.Identity

        @block.sync
        def _(sync):
            # ring-skip dummy: HW-DGE rings are assigned per-engine from the
            # same base, so without it this engine's first DMA would share a
            # ring with ScalarE's xmb half and serialize behind it
            hoist.append(
                sync.dma_start(out=scratch_bf[:], in_=xmb[0:1, 0:1])
                .then_inc(dma_x, 16)
                .ins
            )
            hoist.append(
                sync.dma_start(out=xm_t[14:KROWS, :], in_=xmb[14:KROWS, :])
                .then_inc(dma_x, 16)
                .ins
            )
            hoist.append(
                sync.dma_start(out=xm_t[0:4, :], in_=xmb[0:4, :])
                .then_inc(dma_x, 16)
                .ins
            )
            sync.wait_ge(s_dve, 4)
            sync.dma_start(out=out[:], in_=out_t[:]).then_inc(dma_x, 16)

        @block.gpsimd
        def _(gpsimd):
            # SW-DGE queue: parallel to the HW-DGE rings. ScalarE issues no
            # DMA at all: its ACT table load needs an engine drain, so any
            # in-flight ScalarE DMA would stall the table load ~1.5us.
            hoist.append(
                gpsimd.dma_start(out=xm_t[4:14, :], in_=xmb[4:14, :])
                .then_inc(dma_x, 16)
                .ins
            )
            hoist.append(
                gpsimd.dma_start(out=lw_t[:], in_=lw[:]).then_inc(dma_l, 16).ins
            )

        @block.scalar
        def _(scalar):
            # dummy Sin: forces the ACT table load before the real Sins.
            # Kept in this block (not hoisted): walrus tracks table
            # residency per basic block and would reload after the branch.
            scalar.activation(scratch[:], scratch[:], sin_f, bias=scratch[:])
            scalar.wait_ge(dma_l, 16)  # bias columns used below
            # ScalarE is idle mid-kernel: it computes the second half's
            # k = round(arg/2pi) itself (magic-number trick via Identity)
            # so DVE only has the final fused multiply-add for that half
            scalar.wait_ge(s_pe, 2)
            scalar.activation(
                kred[:, SPLIT:ncols], arg[:, BOFF + SPLIT : BOFF + ncols], ident,
                scale=INV2PI, bias=magic_col,
            )
            scalar.activation(
                kred[:, SPLIT:ncols], kred[:, SPLIT:ncols], ident, bias=nmagic_col
            ).then_inc(s_act, 1)
            for h, (c0, c1) in enumerate(halves):
                scalar.wait_ge(s_dve, 2 + h)
                scalar.activation(
                    feat[:, c0:c1], rred[:, c0:c1], sin_f, bias=zeros_col
                ).then_inc(s_act, 1)

        @block.tensor
        def _(tensor):
            tensor.wait_ge(dma_x, 64)  # ring-skip dummy + three xmb chunks
            # phase arguments: arg[p, (b,j)] = w_j * x_{b,p} + phi_j
            tensor.matmul(
                arg[:, 0:SPLIT],
                xm_t[:, 0:128],
                xm_t[:, 128 : 128 + SPLIT],
                start=True,
                stop=True,
            ).then_inc(s_pe, 1)
            tensor.matmul(
                arg[:, BOFF + SPLIT : BOFF + ncols],
                xm_t[:, 0:128],
                xm_t[:, 128 + SPLIT : 128 + ncols],
                start=True,
                stop=True,
            ).then_inc(s_pe, 1)
            tensor.wait_ge(s_act, 2)
            # contract points: red[w, j] += sum_p w6[p, w*13+b] * feat[p, b*NFEAT+j]
            for b in range(7):
                tensor.matmul(
                    red[:],
                    w6[:, b : b + (NW - 1) * BLOCKS + 1 : BLOCKS],
                    feat[:, b * NFEAT : (b + 1) * NFEAT],
                    start=(b == 0),
                    stop=False,
                )
            tensor.wait_ge(s_act, 3)
            for b in range(7, BLOCKS):
                mm = tensor.matmul(
                    red[:],
                    w6[:, b : b + (NW - 1) * BLOCKS + 1 : BLOCKS],
                    feat[:, b * NFEAT : (b + 1) * NFEAT],
                    start=False,
                    stop=(b == BLOCKS - 1),
                )
            mm.then_inc(s_pe, 1)

        @block.vector
        def _(vector):
            # weight-mask columns first: lw lands on its own DMA queue well
            # before the feature matmul finishes, so this hides entirely.
            # mask * exact-bf16 weight is exactly representable in bf16.
            vector.wait_ge(dma_l, 16)
            lab_t = lw_t[:, 0:BLOCKS]
            whi_t = lw_t[:, BLOCKS : 2 * BLOCKS]
            wlo_t = lw_t[:, 2 * BLOCKS : 3 * BLOCKS]
            # weight cols per block: [pw_hi pw_lo nw_hi nw_lo m1]
            g = lambda i: w6[:, i * BLOCKS : (i + 1) * BLOCKS]
            vector.scalar_tensor_tensor(g(0), lab_t, 1.0, whi_t, op0=eq, op1=mult)
            vector.scalar_tensor_tensor(g(1), lab_t, 1.0, wlo_t, op0=eq, op1=mult)
            vector.scalar_tensor_tensor(g(2), lab_t, 0.0, whi_t, op0=eq, op1=mult)
            vector.scalar_tensor_tensor(g(3), lab_t, 0.0, wlo_t, op0=eq, op1=mult)
            vector.tensor_scalar(g(4), lab_t, 1.0, None, op0=eq).then_inc(s_dve, 1)
            # range-reduce phases into [-pi, pi]: the HW Sin spline is
            # only valid there. k = round(arg/2pi) via magic-number trick.
            # First half fully on DVE; second half's k comes from ScalarE.
            vector.wait_ge(s_pe, 1)
            vector.tensor_scalar(
                kred[:, 0:SPLIT], arg[:, 0:SPLIT], INV2PI, MAGIC, op0=mult, op1=alu.add
            )
            vector.tensor_scalar(
                kred[:, 0:SPLIT], kred[:, 0:SPLIT], MAGIC, None, op0=alu.subtract
            )
            vector.scalar_tensor_tensor(
                rred[:, 0:SPLIT], kred[:, 0:SPLIT], NEG2PI, arg[:, 0:SPLIT],
                op0=mult, op1=alu.add,
            ).then_inc(s_dve, 1)
            vector.wait_ge(s_act, 1)
            vector.scalar_tensor_tensor(
                rred[:, SPLIT:ncols], kred[:, SPLIT:ncols], NEG2PI,
                arg[:, BOFF + SPLIT : BOFF + ncols],
                op0=mult, op1=alu.add,
            ).then_inc(s_dve, 1)
            vector.wait_ge(s_pe, 3)
            vector.tensor_copy(out_t[:], red[:]).then_inc(s_dve, 1)

    if os.environ.get("KERNEL_NO_SURGERY") != "1":
        _preamble_surgery(nc, hoist)
    return nc


def _preamble_surgery(nc, hoist):
    """Move the input DMAs + dummy activation ahead of the framework's
    start drain/barrier in the entry block, then drop that drain/barrier
    (all cross-engine ordering is carried by explicit semaphores)."""
    f = nc.m.functions[0]
    entry = f.blocks[0]
    # remove the hoisted instructions from their engine blocks
    for blk in f.blocks[1:]:
        drop = [
            i
            for i, inst in enumerate(blk.instructions)
            if any(inst is h for h in hoist)
        ]
        for i in reversed(drop):
            del blk.instructions[i]
    # drop the start drain + all-engine barrier (entry block only)
    drop = [
        i
        for i, inst in enumerate(entry.instructions)
        if type(inst).__name__ in ("InstDrain", "InstEventSemaphore")
    ]
    for i in reversed(drop):
        del entry.instructions[i]
    # drop the exit all-engine barrier events too; keep the exit drains
    # (the SP drain retires the output DMA before the NEFF completes)
    tail = f.blocks[-1]
    drop = [
        i
        for i, inst in enumerate(tail.instructions)
        if type(inst).__name__ == "InstEventSemaphore"
    ]
    for i in reversed(drop):
        del tail.instructions[i]
    # insert hoisted instructions right after the entry call marker
    for i, inst in enumerate(hoist):
        entry.instructions.insert(1 + i, inst)


def _shard_inputs(preds, sample_weights, labels):
    """Build per-core input maps. Layout transforms + lossless bf16
    splitting only (no math beyond the f32 cast of labels)."""
    p = np.ascontiguousarray(preds, dtype=np.float32).reshape(-1)
    lab = np.ascontiguousarray(labels).reshape(-1).astype(np.float32)
    wfull = np.repeat(
        np.ascontiguousarray(sample_weights, dtype=np.float32), C
    ).astype(np.float64)  # per-element sample weight
    w_hi = wfull.astype(_bf).astype(np.float32)
    w_lo = (wfull - w_hi.astype(np.float64)).astype(_bf).astype(np.float32)
    mf = _mfeat_const()
    ncols = BLOCKS * NFEAT

    in_maps = []
    for c in range(NCORES):
        sl = slice(c * CHUNK, (c + 1) * CHUNK)
        xpad = np.zeros(PAD, dtype=np.float64)
        xpad[:CHUNK] = p[sl]
        hi, lo = _bf16_split2(xpad)
        xmb = np.zeros((KROWS, 128 + BLOCKS * NFEAT), dtype=_bf)
        xmb[0:BLOCKS, :128] = hi.reshape(BLOCKS, 128)
        xmb[BLOCKS : 2 * BLOCKS, :128] = lo.reshape(BLOCKS, 128)
        xmb[2 * BLOCKS : 2 * BLOCKS + 2, :128] = 1.0  # ones rows -> phases
        xmb[:, 128:] = mf

        lpad = np.full(PAD, -1.0, dtype=np.float32)  # pad label -1: not pos/neg
        lpad[:CHUNK] = lab[sl]
        whpad = np.zeros(PAD, dtype=np.float32)
        whpad[:CHUNK] = w_hi[sl]
        wlpad = np.zeros(PAD, dtype=np.float32)
        wlpad[:CHUNK] = w_lo[sl]
        lwm = np.concatenate(
            [
                lpad.reshape(BLOCKS, 128).T,
                whpad.reshape(BLOCKS, 128).T,
                wlpad.reshape(BLOCKS, 128).T,
                np.zeros((128, 1), dtype=np.float32),
                np.full((128, 1), MAGIC, dtype=np.float32),
                np.full((128, 1), -MAGIC, dtype=np.float32),
            ],
            axis=1,
        ).astype(_bf)

        in_maps.append({"xmb": xmb, "lw": np.ascontiguousarray(lwm)})
    return in_maps


def _combine(partials):
    """Sum per-core [6, 64] feature sums and apply the bilinear combine."""
    s = np.zeros((NW, NFEAT), dtype=np.float64)
    for part in partials:
        s += part.astype(np.float64)
    spw = s[0] + s[1]                  # pos-weighted feature sums (hi+lo)
    snw = s[2] + s[3]                  # neg-weighted feature sums
    cp, sp = spw[:F], spw[F:]
    cn, sn = snw[:F], snw[F:]
    n_pos = s[4, 0]                    # mask1 . cos(0*x) = count(label==1)
    n_neg = N - n_pos                  # labels are {0,1}
    total = np.sum(
        _COEF_A * (cn * cp + sn * sp) + _COEF_B * (sn * cp - cn * sp)
    )
    return np.asarray(total / (n_pos * n_neg), dtype=np.float32)


def run_on_device(preds, sample_weights, labels, trace=False, **spmd_kwargs):
    """Shard, run the SPMD kernel on cores 0-7, return (result, BassKernelResults)."""
    global _NC_CACHE
    if _NC_CACHE is None:
        _NC_CACHE = _build_nc()
    in_maps = _shard_inputs(preds, sample_weights, labels)
    res = run_bass_kernel_spmd(
        _NC_CACHE, in_maps, core_ids=list(range(NCORES)), trace=trace, **spmd_kwargs
    )
    partials = [res.results[i]["out"] for i in range(NCORES)]
    return _combine(partials), res


def kernel(preds, sample_weights, labels):
    result, _ = run_on_device(preds, sample_weights, labels)
    return result


# revision 58
# speedup vs baseline: 1.1998x; 1.1998x over previous
"""AUCLoss kernel for 8 TRN2 NeuronCores.

Math: loss = sum_{i,j} pw_i * nw_j * softplus(p_j - p_i) / (n_pos * n_neg)
where pw/nw are per-element weights masked to label==1 / label==0.

Instead of materializing the N x N (13184^2) pairwise matrix, we use a
truncated Fourier expansion of softplus on the diff domain [-12, 12]:

    softplus(x) ~= sum_k a_k cos(w_k x) + b_k sin(w_k x)

which separates over pairs (x = n - p):
    cos(w(n-p)) = cos(wn)cos(wp) + sin(wn)sin(wp)
    sin(w(n-p)) = sin(wn)cos(wp) - cos(wn)sin(wp)

So the whole pairwise sum reduces to weighted Fourier feature sums
    C[w,k] = sum_i w_i cos(w_k p_i),  S[w,k] = sum_i w_i sin(w_k p_i)
for weight vectors in {pos-weight, neg-weight, mask1, mask0}, then a tiny
bilinear combine. The frequencies are snapped to exact-bf16 values and the
coefficients are least-squares fit for those exact frequencies.

Each core processes 1648 points (13 blocks of 128):
  - two bf16 matmuls against a constant block-diagonal frequency matrix
    build all phase arguments w_k * x + phi (phi = pi/2 turns sin into
    cos), one matmul per PSUM bank so DVE and ScalarE never read the
    same bank concurrently (that hangs the core). x is passed as an
    exact 2-way bf16 split (hi/lo) so the bf16 matmul reproduces
    near-fp32 phases in a single pass instead of fp32's LOW/HIGH
    double pass.
  - the hardware Sin spline is only valid on [-pi, pi] (measured:
    garbage beyond |x|~3.2), so the phases are range-reduced first:
    k = round(arg/2pi) via the fp32 magic-number trick, r = arg - 2pi*k.
    Chunk A runs on DVE; chunk B's k comes from ScalarE Identity
    activations so the two engines split the work.
  - two Sin activations produce all 2F=32 bf16 features per point
  - 5 DVE ops build [pw_hi, pw_lo, nw_hi, nw_lo, m1] weight columns
    from a host-side exact bf16 hi/lo split of the sample weights (mask
    products are exactly representable, so the weighted sums are
    fp32-accurate after the hi+lo recombine on the host)
  - 13 accumulating bf16 matmuls contract points -> [5, 32] partial sums
Host sums the 8 per-core [5, 32] partials and applies the combine
(n_neg = N - n_pos since labels are {0,1}).
"""

import os

import numpy as np
import ml_dtypes

import concourse.bass as bass
import concourse.mybir as mybir
from concourse.bass_utils import run_bass_kernel_spmd

# ---------------------------------------------------------------- constants
B, C = 64, 206
N = B * C                      # 13184 flattened preds
NCORES = 8
CHUNK = N // NCORES            # 1648 points per core
BLOCKS = 13                    # ceil(1648 / 128)
PAD = BLOCKS * 128             # 1664 (16 zero-pad points per core)
F = 16                         # frequencies
NFEAT = 2 * F                  # 32 features per point (cos block | sin block)
PERIOD = 12.5                  # nominal half-period of the Fourier basis
FIT_X = 9.0                    # fit domain for softplus diffs
KROWS = 2 * BLOCKS + 2         # 28 lhsT rows: x_hi, x_lo blocks + 2 ones rows
NW = 5                         # weight columns per block (no mask0: n_neg = N - n_pos)
LWCOLS = 3 * BLOCKS + 3        # labels | w_hi | w_lo | zeros | MAGIC | -MAGIC

MAGIC = 12582912.0             # 1.5 * 2^23: fp32 round-to-nearest-int trick
INV2PI = float(np.float32(1.0 / (2 * np.pi)))
NEG2PI = -float(np.float32(2 * np.pi))

_f32 = mybir.dt.float32
_bf16 = mybir.dt.bfloat16
_bf = ml_dtypes.bfloat16


def _bf16_split2(x):
    """Exact 2-way bf16 split: x ~= hi + lo to ~2^-18 relative."""
    x = np.asarray(x, dtype=np.float64)
    hi = x.astype(_bf)
    lo = (x - hi.astype(np.float64)).astype(_bf)
    return hi, lo


def _fit_fourier():
    """Least-squares fit softplus(x) on [-FIT_X, FIT_X] in the basis
    {cos(w_k x), sin(w_k x)} with w_k snapped to exact bf16 values."""
    w = (np.arange(F) * np.pi / PERIOD).astype(_bf).astype(np.float64)
    xs = np.linspace(-FIT_X, FIT_X, 8001)
    A = np.concatenate(
        [np.cos(np.outer(xs, w)), np.sin(np.outer(xs, w))], axis=1
    )
    y = np.log1p(np.exp(-np.abs(xs))) + np.maximum(xs, 0.0)
    coef = np.linalg.solve(A.T @ A + 1e-9 * np.eye(2 * F), A.T @ y)
    return w, coef[:F], coef[F:]


_OMEGA, _COEF_A, _COEF_B = _fit_fourier()


def _mfeat_const():
    """[KROWS, BLOCKS*NFEAT] bf16 frequency matrix.

    Column b*NFEAT + j: frequency _OMEGA[j % F] in rows b and 13+b
    (multiplying x_hi, x_lo of block b), phase pi/2 (2-way bf16 split
    in the two ones-rows) for j < F (cos features), 0 for sin.
    """
    m = np.zeros((KROWS, BLOCKS * NFEAT), dtype=np.float64)
    for b in range(BLOCKS):
        for r in (b, BLOCKS + b):
            m[r, b * NFEAT : b * NFEAT + F] = _OMEGA
            m[r, b * NFEAT + F : (b + 1) * NFEAT] = _OMEGA
    ph_hi = _bf(np.pi / 2)
    ph_lo = _bf(np.pi / 2 - float(ph_hi))
    for b in range(BLOCKS):
        m[2 * BLOCKS, b * NFEAT : b * NFEAT + F] = float(ph_hi)
        m[2 * BLOCKS + 1, b * NFEAT : b * NFEAT + F] = float(ph_lo)
    return m.astype(_bf)


_NC_CACHE = None


def _build_nc():
    # Raw Bass (no TileContext): the pipeline is a short linear chain and
    # explicit semaphores avoid both the per-instruction sync-wait slot
    # limits and Tile's multi-microsecond kernel-tail drain/barrier.
    #
    # Scheduling notes:
    # - the two input DMAs and a dummy Sin (which forces the ~2.7us ACT
    #   table load early) are hoisted into the entry block, ahead of the
    #   framework's start drain+barrier, so DMA latency and the table
    #   load overlap the fixed engine-startup preamble
    # - the start drain+barrier pair is removed: all cross-engine
    #   ordering is carried by this kernel's explicit semaphores
    # - everything is processed in two halves (PSUM banks of 512/320
    #   cols) so PE, DVE and ACT pipeline against each other
    nc = bass.Bass(enable_partition_id=False, monotonic_sem_count=0)
    ncols = BLOCKS * NFEAT  # 416: fits one PSUM bank, one feature matmul
    # xmb: cols 0:128 = x hi/mid/lo blocks + ones rows, cols 128: = freqs
    xmb = nc.declare_dram_parameter("xmb", [KROWS, 128 + ncols], _bf16, isOutput=False)
    lw = nc.declare_dram_parameter("lw", [128, LWCOLS], _bf16, isOutput=False)
    out = nc.declare_dram_parameter("out", [NW, NFEAT], _f32, isOutput=True)

    alu = mybir.AluOpType
    eq = alu.is_equal
    mult = alu.mult
    sin_f = mybir.ActivationFunctionType.Sin
    hoist = []
    # chunk A = blocks 0-6, chunk B = blocks 7-12. Chunk B's phase args go
    # to a second PSUM bank (col offset 512): ACT reads bank 1 while DVE
    # reads bank 0 -- concurrent cross-engine reads of one PSUM bank hang.
    SPLIT = 7 * NFEAT  # 224
    BOFF = 512 - SPLIT  # bank-1 offset of chunk B inside the psum tensor
    halves = [(0, SPLIT), (SPLIT, ncols)]

    with (
        nc.sbuf_tensor([KROWS, 128 + ncols], _bf16) as xm_t,
        nc.sbuf_tensor([128, LWCOLS], _bf16) as lw_t,
        nc.sbuf_tensor([128, ncols], _f32) as kred,
        nc.sbuf_tensor([128, ncols], _f32) as rred,
        nc.sbuf_tensor([128, ncols], _bf16) as feat,
        nc.sbuf_tensor([128, NW * BLOCKS], _bf16) as w6,
        nc.sbuf_tensor([128, 1], _f32) as scratch,
        nc.sbuf_tensor([1, 1], _bf16) as scratch_bf,
        nc.sbuf_tensor([NW, NFEAT], _f32) as out_t,
        nc.psum_tensor([128, 512 + ncols - SPLIT], _f32) as arg,
        nc.psum_tensor([NW, NFEAT], _f32) as red,
        nc.semaphore() as dma_x,
        nc.semaphore() as dma_l,
        nc.semaphore() as s_pe,
        nc.semaphore() as s_act,
        nc.semaphore() as s_dve,
        nc.Block(no_gpsimd_drain=True) as block,
    ):
        zeros_col = lw_t[:, 3 * BLOCKS : 3 * BLOCKS + 1]
        magic_col = lw_t[:, 3 * BLOCKS + 1 : 3 * BLOCKS + 2]
        nmagic_col = lw_t[:, 3 * BLOCKS + 2 : 3 * BLOCKS + 3]
        ident = mybir.ActivationFunctionType.Identity

        @block.sync
        def _(sync):
            # ring-skip dummy: HW-DGE rings are assigned per-engine from the
            # same base, so without it this engine's first DMA would share a
            # ring with ScalarE's xmb half and serialize behind it
            hoist.append(
                sync.dma_start(out=scratch_bf[:], in_=xmb[0:1, 0:1])
                .then_inc(dma_x, 16)
                .ins
            )
            hoist.append(
                sync.dma_start(out=xm_t[14:KROWS, :], in_=xmb[14:KROWS, :])
                .then_inc(dma_x, 16)
                .ins
            )
            sync.wait_ge(s_dve, 4)
            sync.dma_start(out=out[:], in_=out_t[:]).then_inc(dma_x, 16)

        @block.gpsimd
        def _(gpsimd):
            # SW-DGE queue: parallel to the HW-DGE rings. ScalarE issues no
            # DMA at all: its ACT table load needs an engine drain, so any
            # in-flight ScalarE DMA would stall the table load ~1.5us.
            hoist.append(
                gpsimd.dma_start(out=xm_t[0:14, :], in_=xmb[0:14, :])
                .then_inc(dma_x, 16)
                .ins
            )
            hoist.append(
                gpsimd.dma_start(out=lw_t[:], in_=lw[:]).then_inc(dma_l, 16).ins
            )

        @block.scalar
        def _(scalar):
            # dummy Sin: forces the ACT table load before the real Sins.
            # Kept in this block (not hoisted): walrus tracks table
            # residency per basic block and would reload after the branch.
            scalar.activation(scratch[:], scratch[:], sin_f, bias=scratch[:])
            scalar.wait_ge(dma_l, 16)  # bias columns used below
            # ScalarE is idle mid-kernel: it computes the second half's
            # k = round(arg/2pi) itself (magic-number trick via Identity)
            # so DVE only has the final fused multiply-add for that half
            scalar.wait_ge(s_pe, 2)
            scalar.activation(
                kred[:, SPLIT:ncols], arg[:, BOFF + SPLIT : BOFF + ncols], ident,
                scale=INV2PI, bias=magic_col,
            )
            scalar.activation(
                kred[:, SPLIT:ncols], kred[:, SPLIT:ncols], ident, bias=nmagic_col
            ).then_inc(s_act, 1)
            for h, (c0, c1) in enumerate(halves):
                scalar.wait_ge(s_dve, 1 + h)
                scalar.activation(
                    feat[:, c0:c1], rred[:, c0:c1], sin_f, bias=zeros_col
                ).then_inc(s_act, 1)

        @block.tensor
        def _(tensor):
            tensor.wait_ge(dma_x, 48)  # ring-skip dummy + both xmb chunks
            # phase arguments: arg[p, (b,j)] = w_j * x_{b,p} + phi_j
            tensor.matmul(
                arg[:, 0:SPLIT],
                xm_t[:, 0:128],
                xm_t[:, 128 : 128 + SPLIT],
                start=True,
                stop=True,
            ).then_inc(s_pe, 1)
            tensor.matmul(
                arg[:, BOFF + SPLIT : BOFF + ncols],
                xm_t[:, 0:128],
                xm_t[:, 128 + SPLIT : 128 + ncols],
                start=True,
                stop=True,
            ).then_inc(s_pe, 1)
            tensor.wait_ge(s_act, 2)
            tensor.wait_ge(s_dve, 3)  # weight-mask columns ready
            # contract points: red[w, j] += sum_p w6[p, w*13+b] * feat[p, b*NFEAT+j]
            for b in range(7):
                tensor.matmul(
                    red[:],
                    w6[:, b : b + (NW - 1) * BLOCKS + 1 : BLOCKS],
                    feat[:, b * NFEAT : (b + 1) * NFEAT],
                    start=(b == 0),
                    stop=False,
                )
            tensor.wait_ge(s_act, 3)
            for b in range(7, BLOCKS):
                mm = tensor.matmul(
                    red[:],
                    w6[:, b : b + (NW - 1) * BLOCKS + 1 : BLOCKS],
                    feat[:, b * NFEAT : (b + 1) * NFEAT],
                    start=False,
                    stop=(b == BLOCKS - 1),
                )
            mm.then_inc(s_pe, 1)

        @block.vector
        def _(vector):
            # range reduction first (it gates the Sins -> the critical
            # path); the weight-mask columns only gate the reduction
            # matmuls, so they fit in DVE's idle window afterwards.
            # k = round(arg/2pi) via magic-number trick; the HW Sin
            # spline is only valid on [-pi, pi].
            vector.wait_ge(s_pe, 1)
            vector.tensor_scalar(
                kred[:, 0:SPLIT], arg[:, 0:SPLIT], INV2PI, MAGIC, op0=mult, op1=alu.add
            )
            vector.tensor_scalar(
                kred[:, 0:SPLIT], kred[:, 0:SPLIT], MAGIC, None, op0=alu.subtract
            )
            vector.scalar_tensor_tensor(
                rred[:, 0:SPLIT], kred[:, 0:SPLIT], NEG2PI, arg[:, 0:SPLIT],
                op0=mult, op1=alu.add,
            ).then_inc(s_dve, 1)
            vector.wait_ge(s_act, 1)
            vector.scalar_tensor_tensor(
                rred[:, SPLIT:ncols], kred[:, SPLIT:ncols], NEG2PI,
                arg[:, BOFF + SPLIT : BOFF + ncols],
                op0=mult, op1=alu.add,
            ).then_inc(s_dve, 1)
            # weight cols per block: [pw_hi pw_lo nw_hi nw_lo m1];
            # mask * exact-bf16 weight is exactly representable in bf16
            vector.wait_ge(dma_l, 16)
            lab_t = lw_t[:, 0:BLOCKS]
            whi_t = lw_t[:, BLOCKS : 2 * BLOCKS]
            wlo_t = lw_t[:, 2 * BLOCKS : 3 * BLOCKS]
            g = lambda i: w6[:, i * BLOCKS : (i + 1) * BLOCKS]
            vector.scalar_tensor_tensor(g(0), lab_t, 1.0, whi_t, op0=eq, op1=mult)
            vector.scalar_tensor_tensor(g(1), lab_t, 1.0, wlo_t, op0=eq, op1=mult)
            vector.scalar_tensor_tensor(g(2), lab_t, 0.0, whi_t, op0=eq, op1=mult)
            vector.scalar_tensor_tensor(g(3), lab_t, 0.0, wlo_t, op0=eq, op1=mult)
            vector.tensor_scalar(g(4), lab_t, 1.0, None, op0=eq).then_inc(s_dve, 1)
            vector.wait_ge(s_pe, 3)
            vector.tensor_copy(out_t[:], red[:]).then_inc(s_dve, 1)

    if os.environ.get("KERNEL_NO_SURGERY") != "1":
        _preamble_surgery(nc, hoist)
    return nc


def _preamble_surgery(nc, hoist):
    """Move the input DMAs + dummy activation ahead of the framework's
    start drain/barrier in the entry block, then drop that drain/barrier
    (all cross-engine ordering is carried by explicit semaphores)."""
    f = nc.m.functions[0]
    entry = f.blocks[0]
    # remove the hoisted instructions from their engine blocks
    for blk in f.blocks[1:]:
        drop = [
            i
            for i, inst in enumerate(blk.instructions)
            if any(inst is h for h in hoist)
        ]
        for i in reversed(drop):
            del blk.instructions[i]
    # drop the start drain + all-engine barrier (entry block only)
    drop = [
        i
        for i, inst in enumerate(entry.instructions)
        if type(inst).__name__ in ("InstDrain", "InstEventSemaphore")
    ]
    for i in reversed(drop):
        del entry.instructions[i]
    # drop the exit all-engine barrier events too; keep the exit drains
    # (the SP drain retires the output DMA before the NEFF completes)
    tail = f.blocks[-1]
    drop = [
        i
        for i, inst in enumerate(tail.instructions)
        if type(inst).__name__ == "InstEventSemaphore"
    ]
    for i in reversed(drop):
        del tail.instructions[i]
    # insert hoisted instructions right after the entry call marker
    for i, inst in enumerate(hoist):
        entry.instructions.insert(1 + i, inst)


def _shard_inputs(preds, sample_weights, labels):
    """Build per-core input maps. Layout transforms + lossless bf16
    splitting only (no math beyond the f32 cast of labels)."""
    p = np.ascontiguousarray(preds, dtype=np.float32).reshape(-1)
    lab = np.ascontiguousarray(labels).reshape(-1).astype(np.float32)
    wfull = np.repeat(
        np.ascontiguousarray(sample_weights, dtype=np.float32), C
    ).astype(np.float64)  # per-element sample weight
    w_hi = wfull.astype(_bf).astype(np.float32)
    w_lo = (wfull - w_hi.astype(np.float64)).astype(_bf).astype(np.float32)
    mf = _mfeat_const()
    ncols = BLOCKS * NFEAT

    in_maps = []
    for c in range(NCORES):
        sl = slice(c * CHUNK, (c + 1) * CHUNK)
        xpad = np.zeros(PAD, dtype=np.float64)
        xpad[:CHUNK] = p[sl]
        hi, lo = _bf16_split2(xpad)
        xmb = np.zeros((KROWS, 128 + BLOCKS * NFEAT), dtype=_bf)
        xmb[0:BLOCKS, :128] = hi.reshape(BLOCKS, 128)
        xmb[BLOCKS : 2 * BLOCKS, :128] = lo.reshape(BLOCKS, 128)
        xmb[2 * BLOCKS : 2 * BLOCKS + 2, :128] = 1.0  # ones rows -> phases
        xmb[:, 128:] = mf

        lpad = np.full(PAD, -1.0, dtype=np.float32)  # pad label -1: not pos/neg
        lpad[:CHUNK] = lab[sl]
        whpad = np.zeros(PAD, dtype=np.float32)
        whpad[:CHUNK] = w_hi[sl]
        wlpad = np.zeros(PAD, dtype=np.float32)
        wlpad[:CHUNK] = w_lo[sl]
        lwm = np.concatenate(
            [
                lpad.reshape(BLOCKS, 128).T,
                whpad.reshape(BLOCKS, 128).T,
                wlpad.reshape(BLOCKS, 128).T,
                np.zeros((128, 1), dtype=np.float32),
                np.full((128, 1), MAGIC, dtype=np.float32),
                np.full((128, 1), -MAGIC, dtype=np.float32),
            ],
            axis=1,
        ).astype(_bf)

        in_maps.append({"xmb": xmb, "lw": np.ascontiguousarray(lwm)})
    return in_maps


def _combine(partials):
    """Sum per-core [6, 64] feature sums and apply the bilinear combine."""
    s = np.zeros((NW, NFEAT), dtype=np.float64)
    for part in partials:
        s += part.astype(np.float64)
    spw = s[0] + s[1]                  # pos-weighted feature sums (hi+lo)
    snw = s[2] + s[3]                  # neg-weighted feature sums
    cp, sp = spw[:F], spw[F:]
    cn, sn = snw[:F], snw[F:]
    n_pos = s[4, 0]                    # mask1 . cos(0*x) = count(label==1)
    n_neg = N - n_pos                  # labels are {0,1}
    total = np.sum(
        _COEF_A * (cn * cp + sn * sp) + _COEF_B * (sn * cp - cn * sp)
    )
    return np.asarray(total / (n_pos * n_neg), dtype=np.float32)


def run_on_device(preds, sample_weights, labels, trace=False, **spmd_kwargs):
    """Shard, run the SPMD kernel on cores 0-7, return (result, BassKernelResults)."""
    global _NC_CACHE
    if _NC_CACHE is None:
        _NC_CACHE = _build_nc()
    in_maps = _shard_inputs(preds, sample_weights, labels)
    res = run_bass_kernel_spmd(
        _NC_CACHE, in_maps, core_ids=list(range(NCORES)), trace=trace, **spmd_kwargs
    )
    partials = [res.results[i]["out"] for i in range(NCORES)]
    return _combine(partials), res


def kernel(preds, sample_weights, labels):
    result, _ = run_on_device(preds, sample_weights, labels)
    return result
